# revision 1
# baseline (speedup 1.0000x reference)
"""Cosformer causal attention (B=1, L=2048, E=512, H=8) on 8 TRN2 NeuronCores.

Sharding: one head per core. Each core computes, for its head h:
  qc = relu(xc @ Wq_h), kc = relu(xc @ Wk_h)   (xc = x * cos(theta_l) row-scaled)
  qs = relu(xs @ Wq_h), ks = relu(xs @ Wk_h)   (xs = x * sin(theta_l))
  v  = (xc @ Wv_h) / cos(theta_l)              (recover unscaled v)
  chunked causal linear attention with combined cos/sin branches:
    A[l,m] = qc_l.kc_m + qs_l.ks_m  (= q'_l.k'_m * cos(th_l - th_m))
    O_l    = sum_{m<=l} A[l,m] * [v_m, 1]      (65th col accumulates the norm)
  out_hT = W2_h^T @ (O / (nrm + eps))^T        [E, L] partial, bf16
Host: out = sum_h out_h^T.T + b_out.

relu(w*q) == w*relu(q) because cos/sin weights are >= 0 on [0, pi/2).
Bias b_qkv is supported by augmenting x with a ones column (extra K=1
contraction chunk); b_out is added on the host during unsharding.
"""

import numpy as np
import ml_dtypes

import concourse.bass as bass
import concourse.mybir as mybir
from concourse.tile import TileContext
from concourse.vector_clock import ScopedClock

BF16 = mybir.dt.bfloat16
F32 = mybir.dt.float32
AF = mybir.ActivationFunctionType
ALU = mybir.AluOpType

B, L, E, H = 1, 2048, 512, 8
D = E // H            # 64 head dim
C = 128               # chunk length
NCH = L // C          # 16 chunks
GRP = 4               # chunks per psum group
NG = NCH // GRP       # 4 groups
LT = 512              # l tile for N=512 matmul streams
NLT = L // LT         # 4
EPS = 1e-6
N_CORES = 8


def _split_multi_waits(bir_json):
    """The walrus in this container accepts at most ONE sem wait per
    instruction; split extras into standalone EventSemaphore waits placed
    immediately before the instruction (same engine => order preserved)."""
    import json as _json

    js = _json.loads(bir_json)
    ctr = 0
    for fn in js.get("functions", []):
        for bb in fn.get("blocks", []):
            insts = bb.get("instructions")
            if not insts:
                continue
            out = []
            changed = False
            for inst in insts:
                si = inst.get("sync_info")
                waits = si.get("on_wait", []) if si else []
                if len(waits) > 1:
                    changed = True
                    for w in waits[:-1]:
                        ctr += 1
                        out.append({
                            "debug": inst.get("debug", 0),
                            "engine": inst["engine"],
                            "ins": [],
                            "name": f"I-splitw-{ctr}",
                            "opcode": "EventSemaphore",
                            "outs": [],
                            "sync_info": {"on_update": [], "on_wait": [w]},
                        })
                    si["on_wait"] = [waits[-1]]
                out.append(inst)
            if changed:
                bb["instructions"] = out
    return _json.dumps(js).encode()


def _install_wait_split_hook():
    import concourse.bass2jax as bass2jax
    import concourse.bass_utils as bass_utils

    if getattr(bass2jax, "_wait_split_installed", False):
        return
    orig = bass_utils.compile_bir_kernel

    def patched(bir_json, tmpdir, neff_name="file.neff"):
        return orig(_split_multi_waits(bir_json), tmpdir, neff_name=neff_name)

    bass2jax.compile_bir_kernel = patched
    bass_utils.compile_bir_kernel = patched
    bass2jax._wait_split_installed = True


_install_wait_split_hook()


class SplitDrainTileContext(TileContext):
    """walrus in this container rejects >1 sem wait on the final SP Drain;
    spread the accumulated waits over single-wait SP wait instructions."""

    def _drain_and_barrier(self, tick_clock, wait_clock):
        nc = self.nc
        drain_inst = nc.sync.drain()
        wait_clock.add_sem_waits(
            drain_inst.ins, ScopedClock({None: tick_clock.global_clock})
        )
        waits = list(drain_inst.ins.sync_info.on_wait)
        if len(waits) > 1:
            drain_inst.ins.sync_info.on_wait = waits[:1]
            name2sem = {v.name: v for v in self.sems.allocated().values()}
            for w in waits[1:]:
                nc.sync.wait_ge(name2sem[w.ant_name], w.wait_value)
        nc.all_engine_barrier()
        popped = nc._tile_sem_poison_stack.pop()
        assert popped is self._sem_poison
        nc.clear_and_free_semaphores(list(self.sems.allocated().values()))
        nc.all_engine_barrier()


def build_program(e_in=E, repeat=1, debug=False):
    """Build the SPMD per-core Bass program.

    e_in: contraction length of x (512, or 513 when b_qkv is nonzero and x
          is augmented with a ones column).
    repeat: unroll the whole body this many times (for timing slopes).
    """
    nc = bass.Bass("TRN2", target_bir_lowering=False, debug=False,
                   num_devices=N_CORES)

    # (index, partition_count) contraction chunks over e_in
    ecs = [(i, 128) for i in range(4)]
    if e_in > 4 * 128:
        assert e_in == 4 * 128 + 1
        ecs.append((4, e_in - 4 * 128))
    NEC = len(ecs)

    xcT = nc.dram_tensor("xcT", [e_in, L], BF16, kind="ExternalInput")
    xsT = nc.dram_tensor("xsT", [e_in, L], BF16, kind="ExternalInput")
    wq = nc.dram_tensor("wq", [e_in, D], BF16, kind="ExternalInput")
    wk = nc.dram_tensor("wk", [e_in, D], BF16, kind="ExternalInput")
    wkv = nc.dram_tensor("wkv", [e_in, 2 * D], BF16, kind="ExternalInput")
    w2 = nc.dram_tensor("w2", [D, E], BF16, kind="ExternalInput")
    maskd = nc.dram_tensor("mask", [C, C], F32, kind="ExternalInput")
    rcosd = nc.dram_tensor("rcos", [C, NCH], F32, kind="ExternalInput")
    tand = nc.dram_tensor("tanv", [C, NCH], F32, kind="ExternalInput")
    identd = nc.dram_tensor("ident", [C, C], BF16, kind="ExternalInput")
    outT = nc.dram_tensor("outT", [E, L], BF16, kind="ExternalOutput")

    dbg = {}
    if debug:
        for nm, shp in [("d_qc", [D, L]), ("d_kc", [D, L]), ("d_qs", [D, L]),
                        ("d_ks", [D, L]), ("d_kcn", [C, NCH * D]),
                        ("d_ksn", [C, NCH * D]), ("d_vaug", [C, NCH * (D + 1)]),
                        ("d_at", [C, NCH * C]), ("d_obf", [C, NCH * D]),
                        ("d_ot", [D, L]), ("d_kvs", [D, NCH * 2 * (D + 1)])]:
            dbg[nm] = nc.dram_tensor(nm, shp, BF16, kind="ExternalOutput")

    with SplitDrainTileContext(nc) as tc:
        with (
            tc.tile_pool(name="const", bufs=1) as cpool,
            tc.tile_pool(name="work", bufs=1) as wpool,
            tc.tile_pool(name="stage", bufs=3) as spool,
            tc.tile_pool(name="pbig", bufs=2, space="PSUM") as pbig,
            tc.tile_pool(name="ps0", bufs=1, space="PSUM") as ps0,
            tc.tile_pool(name="po", bufs=2, space="PSUM") as po,
            tc.tile_pool(name="pot", bufs=1, space="PSUM") as pot,
            tc.tile_pool(name="pkv", bufs=2, space="PSUM") as pkv,
        ):
            # ---- static SBUF tensors (one slot each) ----
            xc_sb = cpool.tile([128, NEC, L], BF16, tag="xc")
            xs_sb = cpool.tile([128, NEC, L], BF16, tag="xs")
            wq_sb = cpool.tile([128, NEC, D], BF16, tag="wq")
            wk_sb = cpool.tile([128, NEC, D], BF16, tag="wk")
            wkv_sb = cpool.tile([128, NEC, 2 * D], BF16, tag="wkv")
            w2_sb = cpool.tile([D, NLT, 128], BF16, tag="w2")
            mask_sb = cpool.tile([C, C], F32, tag="mask")
            rcos_sb = cpool.tile([C, NCH], F32, tag="rcos")
            tan_sb = cpool.tile([C, NCH], F32, tag="tan")
            ident_sb = cpool.tile([C, C], BF16, tag="ident")

            qc_sb = wpool.tile([D, L], BF16, tag="qc")
            kc_sb = wpool.tile([D, L], BF16, tag="kc")
            qs_sb = wpool.tile([D, L], BF16, tag="qs")
            ks_sb = wpool.tile([D, L], BF16, tag="ks")
            kcn_sb = wpool.tile([C, NCH, D], BF16, tag="kcn")
            ksn_sb = wpool.tile([C, NCH, D], BF16, tag="ksn")
            vaug_sb = wpool.tile([C, NCH, D + 1], BF16, tag="vaug")
            at_sb = wpool.tile([C, NCH, C], BF16, tag="at")
            kvsnap_sb = wpool.tile([D, NCH, 2, D + 1], BF16, tag="kvsnap")
            kvf_sb = wpool.tile([D, NCH, 2, D + 1], F32, tag="kvf")
            obf_sb = wpool.tile([C, NCH, D], BF16, tag="obf")
            ot_sb = wpool.tile([D, L], BF16, tag="ot")
            rtmp_sb = wpool.tile([C, NCH], F32, tag="rtmp")
            r_sb = wpool.tile([C, NCH], F32, tag="r")

            # ---- constant DMAs (once, outside the repeat loop) ----
            nc.sync.dma_start(mask_sb[:], maskd[:])
            nc.sync.dma_start(rcos_sb[:], rcosd[:])
            nc.sync.dma_start(tan_sb[:], tand[:])
            nc.sync.dma_start(ident_sb[:], identd[:])
            for t_sb, t_d in [(wq_sb, wq), (wk_sb, wk), (wkv_sb, wkv)]:
                nc.sync.dma_start(
                    t_sb[:, :4, :],
                    t_d[: 4 * 128, :].rearrange("(c p) d -> p c d", p=128),
                )
                if NEC == 5:
                    nc.sync.dma_start(t_sb[:1, 4, :], t_d[4 * 128 :, :])
            nc.sync.dma_start(
                w2_sb[:], w2.rearrange("d (t n) -> d t n", n=128)
            )
            # ones column of v_aug
            nc.gpsimd.memset(vaug_sb[:, :, D : D + 1], 1.0)

            for _rep in range(repeat):
                # ---- x DMAs ----
                for t_sb, t_d in [(xc_sb, xcT), (xs_sb, xsT)]:
                    nc.sync.dma_start(
                        t_sb[:, :4, :],
                        t_d[: 4 * 128, :].rearrange("(c p) l -> p c l", p=128),
                    )
                    if NEC == 5:
                        nc.sync.dma_start(t_sb[:1, 4, :], t_d[4 * 128 :, :])

                # ---- phase 1a: transposed projections qc|kc (from xc), qs|ks (xs)
                # qT[d, l] = sum_e W[e, d] * xT[e, l]
                for src_sb, q_out, k_out in (
                    (xc_sb, qc_sb, kc_sb),
                    (xs_sb, qs_sb, ks_sb),
                ):
                    for lt in range(NLT):
                        pq = pbig.tile([128, LT], F32, tag="big")
                        pk = pbig.tile([128, LT], F32, tag="big")
                        for i, (ec, pc) in enumerate(ecs):
                            rhs = src_sb[:pc, ec, lt * LT : (lt + 1) * LT]
                            nc.tensor.matmul(
                                pq[:D, :], wq_sb[:pc, ec, :], rhs,
                                start=(i == 0), stop=(i == NEC - 1),
                            )
                            nc.tensor.matmul(
                                pk[:D, :], wk_sb[:pc, ec, :], rhs,
                                start=(i == 0), stop=(i == NEC - 1),
                            )
                        # relu evacuations (ACT for q, DVE for k)
                        nc.scalar.activation(
                            q_out[:, lt * LT : (lt + 1) * LT], pq[:D, :], AF.Relu
                        )
                        nc.vector.tensor_scalar_max(
                            k_out[:, lt * LT : (lt + 1) * LT], pk[:D, :], 0.0
                        )

                # ---- phase 1b: natural k|v per chunk: kcn = relu(.), v = vc/cos
                for g in range(NG):
                    pkc = pbig.tile([128, GRP, 128], F32, tag="big")
                    for sub in range(GRP):
                        j = g * GRP + sub
                        for i, (ec, pc) in enumerate(ecs):
                            nc.tensor.matmul(
                                pkc[:, sub, :],
                                xc_sb[:pc, ec, j * C : (j + 1) * C],
                                wkv_sb[:pc, ec, :],
                                start=(i == 0), stop=(i == NEC - 1),
                            )
                    gs = slice(g * GRP, (g + 1) * GRP)
                    nc.scalar.activation(
                        kcn_sb[:, gs, :], pkc[:, :, :D], AF.Relu
                    )
                    nc.vector.tensor_tensor(
                        vaug_sb[:, gs, :D],
                        pkc[:, :, D:],
                        rcos_sb[:, gs, None].to_broadcast([C, GRP, D]),
                        ALU.mult,
                    )
                    nc.vector.tensor_tensor(
                        ksn_sb[:, gs, :],
                        kcn_sb[:, gs, :],
                        tan_sb[:, gs, None].to_broadcast([C, GRP, D]),
                        ALU.mult,
                    )

                # ---- phase 2: chunked causal attention ----
                for g in range(NG):
                    # S0 for the 4 chunks of this group, packed in one bank
                    s0 = ps0.tile([C, GRP, C], F32, tag="s0")
                    for sub in range(GRP):
                        j = g * GRP + sub
                        cs = slice(j * C, (j + 1) * C)
                        nc.tensor.matmul(
                            s0[:, sub, :], kc_sb[:, cs], qc_sb[:, cs],
                            start=True, stop=False,
                        )
                        nc.tensor.matmul(
                            s0[:, sub, :], ks_sb[:, cs], qs_sb[:, cs],
                            start=False, stop=True,
                        )
                    gs = slice(g * GRP, (g + 1) * GRP)
                    nc.vector.tensor_tensor(
                        at_sb[:, gs, :],
                        s0[:],
                        mask_sb[:, None, :].to_broadcast([C, GRP, C]),
                        ALU.mult,
                    )

                    o_ps = po.tile([C, GRP, 128], F32, tag="o")
                    for sub in range(GRP):
                        j = g * GRP + sub
                        cs = slice(j * C, (j + 1) * C)
                        last = j == 0  # chunk 0 has no inter part
                        nc.tensor.matmul(
                            o_ps[:, sub, :D + 1], at_sb[:, j, :], vaug_sb[:, j, :],
                            start=True, stop=last,
                        )
                        if j > 0:
                            nc.tensor.matmul(
                                o_ps[:, sub, :D + 1], qc_sb[:, cs],
                                kvsnap_sb[:, j - 1, 0, :],
                                start=False, stop=False,
                            )
                            nc.tensor.matmul(
                                o_ps[:, sub, :D + 1], qs_sb[:, cs],
                                kvsnap_sb[:, j - 1, 1, :],
                                start=False, stop=True,
                            )
                        # state update for chunk j (accumulating psum)
                        if j < NCH - 1:
                            kv_ps = pkv.tile([D, 2, 128], F32, tag="kv")
                            nc.tensor.matmul(
                                kv_ps[:, 0, :D + 1], kcn_sb[:, j, :],
                                vaug_sb[:, j, :], start=True, stop=True,
                            )
                            nc.tensor.matmul(
                                kv_ps[:, 1, :D + 1], ksn_sb[:, j, :],
                                vaug_sb[:, j, :], start=True, stop=True,
                            )
                            if j == 0:
                                nc.vector.tensor_copy(
                                    kvf_sb[:, 0, :, :], kv_ps[:, :, :D + 1]
                                )
                            else:
                                nc.vector.tensor_tensor(
                                    kvf_sb[:, j, :, :], kvf_sb[:, j - 1, :, :],
                                    kv_ps[:, :, :D + 1], ALU.add,
                                )
                            nc.vector.tensor_copy(
                                kvsnap_sb[:, j, :, :], kvf_sb[:, j, :, :]
                            )

                    # normalize: r = 1/(nrm + eps); obf = O * r (bf16)
                    nc.vector.tensor_scalar_add(
                        rtmp_sb[:, gs], o_ps[:, :, D], EPS
                    )
                    nc.vector.reciprocal(r_sb[:, gs], rtmp_sb[:, gs])
                    nc.vector.tensor_tensor(
                        obf_sb[:, gs, :],
                        o_ps[:, :, :D],
                        r_sb[:, gs, None].to_broadcast([C, GRP, D]),
                        ALU.mult,
                    )
                    # transpose each chunk's O to [D, C] for the out projection
                    for sub in range(GRP):
                        j = g * GRP + sub
                        otp = pot.tile([D, C], BF16, tag="ot")
                        nc.tensor.transpose(otp[:], obf_sb[:, j, :], ident_sb[:])
                        nc.vector.tensor_copy(
                            ot_sb[:, j * C : (j + 1) * C], otp[:]
                        )

                # ---- phase 3: out projection (partial, transposed) ----
                for ns in range(NLT):
                    for lt in range(NLT):
                        op = pbig.tile([128, LT], F32, tag="big")
                        nc.tensor.matmul(
                            op[:], w2_sb[:, ns, :],
                            ot_sb[:, lt * LT : (lt + 1) * LT],
                            start=True, stop=True,
                        )
                        ob = spool.tile([128, LT], BF16, tag="ob")
                        if (ns + lt) % 2 == 0:
                            nc.scalar.activation(ob[:], op[:], AF.Copy)
                        else:
                            nc.vector.tensor_copy(ob[:], op[:])
                        nc.sync.dma_start(
                            outT[ns * 128 : (ns + 1) * 128,
                                 lt * LT : (lt + 1) * LT],
                            ob[:],
                        )

            if debug:
                for nm, sb in [("d_qc", qc_sb), ("d_kc", kc_sb),
                               ("d_qs", qs_sb), ("d_ks", ks_sb)]:
                    nc.sync.dma_start(dbg[nm][:], sb[:])
                nc.sync.dma_start(
                    dbg["d_kcn"][:], kcn_sb.rearrange("p a b -> p (a b)")
                )
                nc.sync.dma_start(
                    dbg["d_ksn"][:], ksn_sb.rearrange("p a b -> p (a b)")
                )
                nc.sync.dma_start(
                    dbg["d_vaug"][:], vaug_sb.rearrange("p a b -> p (a b)")
                )
                nc.sync.dma_start(
                    dbg["d_at"][:], at_sb.rearrange("p a b -> p (a b)")
                )
                nc.sync.dma_start(
                    dbg["d_obf"][:], obf_sb.rearrange("p a b -> p (a b)")
                )
                nc.sync.dma_start(dbg["d_ot"][:], ot_sb[:])
                nc.sync.dma_start(
                    dbg["d_kvs"][:], kvsnap_sb.rearrange("p a b c -> p (a b c)")
                )
    return nc


def prepare_in_maps(x, W_qkv, b_qkv, W_out):
    """Host-side sharding/layout prep. Returns (in_maps, e_in)."""
    x = np.asarray(x, dtype=np.float32).reshape(L, E)
    W_qkv = np.asarray(W_qkv, dtype=np.float32)
    b_qkv = np.asarray(b_qkv, dtype=np.float32)
    W_out = np.asarray(W_out, dtype=np.float32)

    use_bias = bool(np.any(b_qkv))
    if use_bias:
        x_aug = np.concatenate([x, np.ones((L, 1), np.float32)], axis=1)
        W_aug = np.concatenate([W_qkv, b_qkv[None, :]], axis=0)
    else:
        x_aug, W_aug = x, W_qkv
    e_in = x_aug.shape[1]

    pos = np.arange(L, dtype=np.float32)
    theta = (np.pi / 2) * pos / L
    cosw = np.cos(theta).astype(np.float32)
    sinw = np.sin(theta).astype(np.float32)

    bf = ml_dtypes.bfloat16
    xcT = np.ascontiguousarray((x_aug * cosw[:, None]).T).astype(bf)
    xsT = np.ascontiguousarray((x_aug * sinw[:, None]).T).astype(bf)
    mask = np.triu(np.ones((C, C), np.float32))  # mask[m, l] = 1 if m <= l
    rcos = np.ascontiguousarray(
        (1.0 / cosw).reshape(NCH, C).T
    ).astype(np.float32)
    tanv = np.ascontiguousarray(
        (sinw / cosw).reshape(NCH, C).T
    ).astype(np.float32)
    ident = np.eye(C, dtype=np.float32).astype(bf)

    in_maps = []
    for h in range(N_CORES):
        hs = slice(h * D, (h + 1) * D)
        wq_h = np.ascontiguousarray(W_aug[:, hs]).astype(bf)
        wk_h = np.ascontiguousarray(W_aug[:, E + h * D : E + (h + 1) * D]).astype(bf)
        wv_h = W_aug[:, 2 * E + h * D : 2 * E + (h + 1) * D]
        wkv_h = np.ascontiguousarray(
            np.concatenate([W_aug[:, E + h * D : E + (h + 1) * D], wv_h], axis=1)
        ).astype(bf)
        w2_h = np.ascontiguousarray(W_out[hs, :]).astype(bf)
        in_maps.append({
            "xcT": xcT, "xsT": xsT,
            "wq": wq_h, "wk": wk_h, "wkv": wkv_h, "w2": w2_h,
            "mask": mask, "rcos": rcos, "tanv": tanv, "ident": ident,
        })
    return in_maps, e_in


def combine_outputs(results, b_out):
    b_out = np.asarray(b_out, dtype=np.float32)
    acc = np.zeros((E, L), np.float32)
    for r in results:
        acc += np.asarray(r["outT"]).astype(np.float32)
    out = acc.T + b_out[None, :]
    return out.reshape(B, L, E).astype(np.float32)


_PROGRAM_CACHE = {}


def _get_program(e_in):
    if e_in not in _PROGRAM_CACHE:
        _PROGRAM_CACHE[e_in] = build_program(e_in=e_in)
    return _PROGRAM_CACHE[e_in]


def kernel(x, W_qkv, b_qkv, W_out, b_out):
    from concourse.bass_utils import run_bass_kernel_spmd

    in_maps, e_in = prepare_in_maps(x, W_qkv, b_qkv, W_out)
    nc = _get_program(e_in)
    res = run_bass_kernel_spmd(nc, in_maps, core_ids=list(range(N_CORES)))
    return combine_outputs(res.results, b_out)



# revision 8
# speedup vs baseline: 1.9980x; 1.9980x over previous
"""Cosformer causal attention (B=1, L=2048, E=512, H=8) on 8 TRN2 NeuronCores.

Instruction-count-minimized rewrite. Sharding: one head per core.

Math per core (head h), with theta_l = (pi/2) l / L:
  qT = wq_h^T xT, kT = wk_h^T xT, vT = wv_h^T xT            (transposed, [64, L])
  qcs = [relu(q)*cos; relu(q)*sin]  (stacked on partitions, [128, L])
  kcs = [relu(k)*cos; relu(k)*sin]
  A[m, l] = kcs_m . qcs_l = relu(q_l).relu(k_m) cos(th_l - th_m)
  natural [relu(k) | v] via 16 XBAR DMA transposes ([128, L] -> 16x[128,128])
  kk_nat = [relu(k)*cos | relu(k)*sin], vaug = [v, 1]       (natural)
  Superchunks W=512 (4 quads):
    S_quad = kcs_chunk^T qcs_quad  (4 mm, N=512); at = S * tri_mask
    OT_quad [65, 512] = kvsnap_{Q-1}^T qcs_quad + sum_i vaug_i^T at_i
    KV state [128, 65] accumulates in PSUM across quads (chained matmuls)
  out_l = OT[0:64, l] / (OT[64, l] + eps);  outT_h = w2_h^T out  ([512, L] partial)
Host: out = sum_h outT_h^T + b_out.
"""

import numpy as np
import ml_dtypes

import concourse.bass as bass
import concourse.mybir as mybir
from concourse.tile import TileContext
from concourse.vector_clock import ScopedClock

BF16 = mybir.dt.bfloat16
F32 = mybir.dt.float32
AF = mybir.ActivationFunctionType
ALU = mybir.AluOpType

B, L, E, H = 1, 2048, 512, 8
D = E // H            # 64 head dim
C = 128               # chunk length
NCH = L // C          # 16 chunks
W = 512               # superchunk (quad) width
NQ = L // W           # 4 quads
CPQ = W // C          # 4 chunks per quad
EPS = 1e-6
N_CORES = 8


def _split_multi_waits(bir_json):
    """The walrus in this container accepts at most ONE sem wait per
    instruction; split extras into standalone EventSemaphore waits placed
    immediately before the instruction (same engine => order preserved)."""
    import json as _json

    js = _json.loads(bir_json)
    ctr = 0
    for fn in js.get("functions", []):
        for bb in fn.get("blocks", []):
            insts = bb.get("instructions")
            if not insts:
                continue
            out = []
            changed = False
            for inst in insts:
                si = inst.get("sync_info")
                waits = si.get("on_wait", []) if si else []
                if len(waits) > 1:
                    changed = True
                    for w in waits[:-1]:
                        ctr += 1
                        out.append({
                            "debug": inst.get("debug", 0),
                            "engine": inst["engine"],
                            "ins": [],
                            "name": f"I-splitw-{ctr}",
                            "opcode": "EventSemaphore",
                            "outs": [],
                            "sync_info": {"on_update": [], "on_wait": [w]},
                        })
                    si["on_wait"] = [waits[-1]]
                out.append(inst)
            if changed:
                bb["instructions"] = out
    return _json.dumps(js).encode()


def _install_wait_split_hook():
    import concourse.bass2jax as bass2jax
    import concourse.bass_utils as bass_utils

    if getattr(bass2jax, "_wait_split_installed", False):
        return
    orig = bass_utils.compile_bir_kernel

    def patched(bir_json, tmpdir, neff_name="file.neff"):
        return orig(_split_multi_waits(bir_json), tmpdir, neff_name=neff_name)

    bass2jax.compile_bir_kernel = patched
    bass_utils.compile_bir_kernel = patched
    bass2jax._wait_split_installed = True


_install_wait_split_hook()


class SplitDrainTileContext(TileContext):
    """walrus in this container rejects >1 sem wait on the final SP Drain;
    spread the accumulated waits over single-wait SP wait instructions."""

    def _drain_and_barrier(self, tick_clock, wait_clock):
        nc = self.nc
        drain_inst = nc.sync.drain()
        wait_clock.add_sem_waits(
            drain_inst.ins, ScopedClock({None: tick_clock.global_clock})
        )
        waits = list(drain_inst.ins.sync_info.on_wait)
        if len(waits) > 1:
            drain_inst.ins.sync_info.on_wait = waits[:1]
            name2sem = {v.name: v for v in self.sems.allocated().values()}
            for w in waits[1:]:
                nc.sync.wait_ge(name2sem[w.ant_name], w.wait_value)
        nc.all_engine_barrier()
        popped = nc._tile_sem_poison_stack.pop()
        assert popped is self._sem_poison
        nc.clear_and_free_semaphores(list(self.sems.allocated().values()))
        nc.all_engine_barrier()


def build_program(e_in=E, repeat=1):
    """SPMD per-core Bass program. e_in = 512, or 513 with bias row."""
    nc = bass.Bass("TRN2", target_bir_lowering=False, debug=False,
                   num_devices=N_CORES)

    ecs = [(i, 128) for i in range(4)]
    if e_in > 4 * 128:
        assert e_in == 4 * 128 + 1
        ecs.append((4, e_in - 4 * 128))
    NEC = len(ecs)

    xT = nc.dram_tensor("xT", [e_in, L], BF16, kind="ExternalInput")
    wqk = nc.dram_tensor("wqk", [e_in, 2 * D], BF16, kind="ExternalInput")
    wv = nc.dram_tensor("wv", [e_in, D], BF16, kind="ExternalInput")
    w2 = nc.dram_tensor("w2", [D, E], BF16, kind="ExternalInput")
    cs64d = nc.dram_tensor("cs64", [D, 2 * L], F32, kind="ExternalInput")
    csmd = nc.dram_tensor("csm", [C, 2 * NCH], F32, kind="ExternalInput")
    maskd = nc.dram_tensor("mask", [C, CPQ * W], F32, kind="ExternalInput")
    outT = nc.dram_tensor("outT", [E, L], BF16, kind="ExternalOutput")

    with SplitDrainTileContext(nc) as tc:
        with (
            tc.tile_pool(name="const", bufs=1) as cpool,
            tc.tile_pool(name="work", bufs=1) as wpool,
            tc.tile_pool(name="pq", bufs=2, space="PSUM") as pq,      # 2x2 banks
            tc.tile_pool(name="pot", bufs=2, space="PSUM") as pot,    # 2x1 bank
            tc.tile_pool(name="pkv", bufs=1, space="PSUM") as pkv,    # 1 bank
            tc.tile_pool(name="prb", bufs=1, space="PSUM") as prb,    # 1 bank
        ):
            # ---- constant tiles ----
            wqk_sb = cpool.tile([128, NEC, 2 * D], BF16, tag="wqk")
            wv_sb = cpool.tile([128, NEC, D], BF16, tag="wv")
            w2_sb = cpool.tile([D, 4, 128], BF16, tag="w2")
            cs64_sb = cpool.tile([D, 2, L], F32, tag="cs64")
            csm_sb = cpool.tile([C, 2, NCH], F32, tag="csm")
            mask_sb = cpool.tile([C, CPQ, W], F32, tag="mask")

            # ---- work tiles ----
            xT_sb = wpool.tile([128, NEC, L], BF16, tag="xT")
            ones_sb = cpool.tile([1, D], F32, tag="ones")
            qcs_sb = wpool.tile([128, L], BF16, tag="qcs")
            kcs_sb = wpool.tile([128, L], BF16, tag="kcs")
            kvT_sb = wpool.tile([128, L], BF16, tag="kvT")
            knat_sb = wpool.tile([C, NCH, 128], BF16, tag="knat")
            kk_sb = wpool.tile([C, NCH, 2 * D], BF16, tag="kk")
            vaug_sb = wpool.tile([C, NCH, D + 1], BF16, tag="vaug")
            at_sb = wpool.tile([C, CPQ, W], BF16, tag="at")
            kvsnap_sb = wpool.tile([128, NQ - 1, D + 1], BF16, tag="kvsnap")
            otf_sb = wpool.tile([D + 1, NQ, W], F32, tag="otf")
            r_sb = wpool.tile([1, L], F32, tag="r")
            obf_sb = wpool.tile([D, L], BF16, tag="obf")
            ob_sb = wpool.tile([128, 4, L], BF16, tag="ob")

            # ---- constant DMAs (outside the repeat loop) ----
            nc.sync.dma_start(
                wqk_sb[:, :4, :],
                wqk[: 4 * 128, :].rearrange("(c p) d -> p c d", p=128))
            nc.sync.dma_start(
                wv_sb[:, :4, :],
                wv[: 4 * 128, :].rearrange("(c p) d -> p c d", p=128))
            if NEC == 5:
                nc.sync.dma_start(wqk_sb[:1, 4, :], wqk[4 * 128:, :])
                nc.sync.dma_start(wv_sb[:1, 4, :], wv[4 * 128:, :])
            nc.sync.dma_start(w2_sb[:], w2.rearrange("d (t n) -> d t n", n=128))
            nc.sync.dma_start(cs64_sb[:], cs64d.rearrange("d (s l) -> d s l", s=2))
            nc.sync.dma_start(csm_sb[:], csmd.rearrange("p (s j) -> p s j", s=2))
            nc.sync.dma_start(mask_sb[:], maskd.rearrange("p (i l) -> p i l", i=CPQ))
            nc.vector.memset(vaug_sb[:, :, D:D + 1], 1.0)
            nc.vector.memset(ones_sb[:], 1.0)

            for _rep in range(repeat):
                # ---- x DMA ----
                nc.sync.dma_start(
                    xT_sb[:, :4, :],
                    xT[: 4 * 128, :].rearrange("(c p) l -> p c l", p=128))
                if NEC == 5:
                    nc.sync.dma_start(xT_sb[:1, 4, :], xT[4 * 128:, :])

                # ---- phase 1a: vT then [q|k]T projections ----
                # v stream: psum rows 0:63 = vT
                for h in range(2):   # L halves of 1024
                    ps = pq.tile([128, 2, 512], F32, tag="pq")
                    for i, (ec, pc) in enumerate(ecs):
                        for t in range(2):
                            ls = slice(1024 * h + 512 * t, 1024 * h + 512 * (t + 1))
                            nc.tensor.matmul(
                                ps[:D, t, :], wv_sb[:pc, ec, :],
                                xT_sb[:pc, ec, ls],
                                start=(i == 0), stop=(i == NEC - 1))
                    # vT -> kvT rows 64:128 (no relu)
                    nc.vector.tensor_copy(
                        kvT_sb[D:, 1024 * h:1024 * (h + 1)],
                        ps[:D, :, :].rearrange("p a b -> p (a b)"))

                # qk stream: psum rows 0:63 = qT, 64:128 = kT
                for h in range(2):
                    ps = pq.tile([128, 2, 512], F32, tag="pq")
                    for i, (ec, pc) in enumerate(ecs):
                        for t in range(2):
                            ls = slice(1024 * h + 512 * t, 1024 * h + 512 * (t + 1))
                            nc.tensor.matmul(
                                ps[:, t, :], wqk_sb[:pc, ec, :],
                                xT_sb[:pc, ec, ls],
                                start=(i == 0), stop=(i == NEC - 1))
                    hs = slice(1024 * h, 1024 * (h + 1))
                    qps = ps[:D, :, :].rearrange("p a b -> p (a b)")
                    kps = ps[D:, :, :].rearrange("p a b -> p (a b)")
                    # qcs rows: 0:64 = relu(q)*cos, 64:128 = relu(q)*sin
                    nc.vector.scalar_tensor_tensor(
                        qcs_sb[:D, hs], qps, 0.0, cs64_sb[:, 0, hs],
                        ALU.max, ALU.mult)
                    nc.vector.scalar_tensor_tensor(
                        qcs_sb[D:, hs], qps, 0.0, cs64_sb[:, 1, hs],
                        ALU.max, ALU.mult)
                    nc.vector.scalar_tensor_tensor(
                        kcs_sb[:D, hs], kps, 0.0, cs64_sb[:, 0, hs],
                        ALU.max, ALU.mult)
                    nc.vector.scalar_tensor_tensor(
                        kcs_sb[D:, hs], kps, 0.0, cs64_sb[:, 1, hs],
                        ALU.max, ALU.mult)
                    # kvT rows 0:64 = relu(kT)
                    nc.vector.tensor_scalar_max(kvT_sb[:D, hs], kps, 0.0)

                # ---- phase 1b: natural [relu(k) | v] via DMA transposes ----
                for j in range(NCH):
                    nc.sync.dma_start(
                        knat_sb[:, j, :], kvT_sb[:, C * j:C * (j + 1)],
                        transpose=True)
                # kk = [relu(k)*cos_m | relu(k)*sin_m], vaug[:, :, :D] = v
                nc.gpsimd.tensor_tensor(
                    kk_sb[:, :, :D], knat_sb[:, :, :D],
                    csm_sb[:, 0, :, None].to_broadcast([C, NCH, D]), ALU.mult)
                nc.gpsimd.tensor_tensor(
                    kk_sb[:, :, D:], knat_sb[:, :, :D],
                    csm_sb[:, 1, :, None].to_broadcast([C, NCH, D]), ALU.mult)
                nc.gpsimd.tensor_copy(vaug_sb[:, :, :D], knat_sb[:, :, D:])

                # ---- fused superchunk loop: phase 2 + norm + phase 3 ----
                kv_ps = pkv.tile([128, D + 1], F32, tag="kv")
                for Q in range(NQ):
                    qs = slice(W * Q, W * (Q + 1))
                    # KV updates for this quad's chunks (state for quad Q+1)
                    if Q < NQ - 1:
                        for c in range(CPQ):
                            j = CPQ * Q + c
                            nc.tensor.matmul(
                                kv_ps[:], kk_sb[:, j, :], vaug_sb[:, j, :],
                                start=(j == 0), stop=(c == CPQ - 1),
                                skip_group_check=True)
                        nc.scalar.copy(kvsnap_sb[:, Q, :], kv_ps[:])
                    # S for the 4 chunks (2 psum gens of 2 banks)
                    for half in range(2):
                        ps = pq.tile([128, 2, 512], F32, tag="pq")
                        for c in range(2):
                            i = 2 * half + c
                            ks = slice(C * (CPQ * Q + i), C * (CPQ * Q + i + 1))
                            nc.tensor.matmul(
                                ps[:, c, :], kcs_sb[:, ks], qcs_sb[:, qs],
                                start=True, stop=True)
                        nc.vector.tensor_tensor(
                            at_sb[:, 2 * half:2 * half + 2, :], ps[:],
                            mask_sb[:, 2 * half:2 * half + 2, :], ALU.mult)
                    # OT group: inter + 4 intra (chained)
                    ot_ps = pot.tile([D + 1, W], F32, tag="ot")
                    if Q > 0:
                        nc.tensor.matmul(
                            ot_ps[:], kvsnap_sb[:, Q - 1, :], qcs_sb[:, qs],
                            start=True, stop=False)
                    for i in range(CPQ):
                        j = CPQ * Q + i
                        nc.tensor.matmul(
                            ot_ps[:, C * i:], vaug_sb[:, j, :],
                            at_sb[:, i, C * i:],
                            start=(Q == 0 and i == 0), stop=(i == CPQ - 1))
                    nc.scalar.copy(otf_sb[:, Q, :], ot_ps[:])

                    # norm for this quad: r = 1/(nrm+eps), broadcast via K=1
                    # f32 matmul (stays on the tensor queue, no DMA latency)
                    nc.vector.tensor_scalar_add(
                        r_sb[:, qs], otf_sb[D:D + 1, Q, :], EPS)
                    nc.vector.reciprocal(r_sb[:, qs], r_sb[:, qs])
                    rb_ps = prb.tile([D, W], F32, tag="rb")
                    nc.tensor.matmul(rb_ps[:], ones_sb[:], r_sb[:, qs],
                                     start=True, stop=True)
                    nc.vector.tensor_tensor(
                        obf_sb[:, qs], otf_sb[:D, Q, :], rb_ps[:], ALU.mult)

                    # phase 3 for this quad's columns
                    for g in range(2):
                        ps = pq.tile([128, 2, 512], F32, tag="pq")
                        for c in range(2):
                            nt = 2 * g + c
                            nc.tensor.matmul(
                                ps[:, c, :], w2_sb[:, nt, :], obf_sb[:, qs],
                                start=True, stop=True)
                        nc.vector.tensor_copy(
                            ob_sb[:, 2 * g:2 * g + 2, qs], ps[:])

                # ---- output DMAs ----
                for nt in range(4):
                    nc.sync.dma_start(outT[128 * nt:128 * (nt + 1), :],
                                      ob_sb[:, nt, :])
    return nc


def prepare_in_maps(x, W_qkv, b_qkv, W_out):
    """Host-side sharding/layout prep. Returns (in_maps, e_in)."""
    x = np.asarray(x, dtype=np.float32).reshape(L, E)
    W_qkv = np.asarray(W_qkv, dtype=np.float32)
    b_qkv = np.asarray(b_qkv, dtype=np.float32)
    W_out = np.asarray(W_out, dtype=np.float32)

    use_bias = bool(np.any(b_qkv))
    if use_bias:
        x_aug = np.concatenate([x, np.ones((L, 1), np.float32)], axis=1)
        W_aug = np.concatenate([W_qkv, b_qkv[None, :]], axis=0)
    else:
        x_aug, W_aug = x, W_qkv
    e_in = x_aug.shape[1]

    bf = ml_dtypes.bfloat16
    xT = np.ascontiguousarray(x_aug.T).astype(bf)

    pos = np.arange(L, dtype=np.float32)
    theta = (np.pi / 2) * pos / L
    cosw = np.cos(theta).astype(np.float32)
    sinw = np.sin(theta).astype(np.float32)

    cs64 = np.concatenate([
        np.broadcast_to(cosw[None, :], (D, L)),
        np.broadcast_to(sinw[None, :], (D, L))], axis=1)
    cs64 = np.ascontiguousarray(cs64).astype(np.float32)
    csm = np.concatenate([
        cosw.reshape(NCH, C).T, sinw.reshape(NCH, C).T], axis=1)
    csm = np.ascontiguousarray(csm).astype(np.float32)
    # mask[m, i, l] = 1 if 128*i + m <= l  (within a quad), flattened [C, CPQ*W]
    m_idx = np.arange(C)[:, None, None]
    i_idx = np.arange(CPQ)[None, :, None]
    l_idx = np.arange(W)[None, None, :]
    mask = (C * i_idx + m_idx <= l_idx).astype(np.float32).reshape(C, CPQ * W)
    mask = np.ascontiguousarray(mask)

    in_maps = []
    for h in range(N_CORES):
        hs = slice(h * D, (h + 1) * D)
        wq_h = W_aug[:, hs]
        wk_h = W_aug[:, E + h * D:E + (h + 1) * D]
        wv_h = W_aug[:, 2 * E + h * D:2 * E + (h + 1) * D]
        wqk_h = np.ascontiguousarray(
            np.concatenate([wq_h, wk_h], axis=1)).astype(bf)
        w2_h = np.ascontiguousarray(W_out[hs, :]).astype(bf)
        in_maps.append({
            "xT": xT, "wqk": wqk_h,
            "wv": np.ascontiguousarray(wv_h).astype(bf),
            "w2": w2_h, "cs64": cs64, "csm": csm, "mask": mask,
        })
    return in_maps, e_in


def combine_outputs(results, b_out):
    b_out = np.asarray(b_out, dtype=np.float32)
    acc = np.zeros((E, L), np.float32)
    for r in results:
        acc += np.asarray(r["outT"]).astype(np.float32)
    out = acc.T + b_out[None, :]
    return out.reshape(B, L, E).astype(np.float32)


_PROGRAM_CACHE = {}


def _get_program(e_in):
    if e_in not in _PROGRAM_CACHE:
        _PROGRAM_CACHE[e_in] = build_program(e_in=e_in)
    return _PROGRAM_CACHE[e_in]


def kernel(x, W_qkv, b_qkv, W_out, b_out):
    from concourse.bass_utils import run_bass_kernel_spmd

    in_maps, e_in = prepare_in_maps(x, W_qkv, b_qkv, W_out)
    nc = _get_program(e_in)
    res = run_bass_kernel_spmd(nc, in_maps, core_ids=list(range(N_CORES)))
    return combine_outputs(res.results, b_out)
